# revision 42
# baseline (speedup 1.0000x reference)
"""Trainium2 Bass kernel for BHS_GCN: 2x GCNConv + dueling value/advantage heads.

Strategy (8 NeuronCores, single NEFF launch):
  - GCN phase batch-parallel: each core owns B_LOC=4 full graphs.
    Message passing = bulk dma_gather of source-node rows + PE one-hot
    scatter-matmuls into PSUM (edges pre-sorted/packed by dst on host,
    in the quarter-major tile order both layers process, so gathers are
    batched over tile groups with contiguous index ranges).
  - AllToAll reshards the pre-W2 aggregation (agg2, [N,4,128] per core) to
    node-parallel: each core gets its 512-node slice for all 32 batches.
  - W2 matmul + relu run node-sharded; the head contraction is FLIPPED
    (lhsT = h2 [128,32], rhs = headW [128,76]) and col-tiled across the
    four 32-wide PE column groups, so head weights stream unpadded (76
    wide) and ldweights is only 32 columns. Each core reads only its 1/8
    slice of advW/v1W (the dominating memory traffic is read once
    machine-wide).
  - AllReduce of [32,76] partial head sums; the tiny val-MLP and dueling
    combine run redundantly on every core; host takes core 0's output.
  - Everything wide is bf16 (accumulation always fp32 PSUM); rel err vs
    the fp32 reference ~5e-3.
"""

import sys

sys.path.insert(0, "/opt/trn_rl_repo")

import os

import numpy as np
import ml_dtypes

# Precision mode: "bf16" (default: everything big in bf16) or "f32"
# (debug; slower, exact). Accumulation is always fp32 PSUM.
PRECISION = os.environ.get("GCN_PREC", "bf16")
BF16 = np.dtype(ml_dtypes.bfloat16)
BF = PRECISION != "f32"

# ---------------- problem constants (hardcoded per contract) ----------------
B, N, F_IN, E = 32, 4096, 16, 16384
NC_CORES = 8
B_LOC = B // NC_CORES            # 4
NSLICE = N // NC_CORES           # 512 nodes per core for head phase
F1, F2 = 128, 256
P = 128
NTILES = N // P                  # 32 node tiles
BF1 = B_LOC * F_IN               # 64   (mp1 row width)
BFH = B_LOC * F1                 # 512  (H1 row width = mp2 gather width)
KTOT = NSLICE * F2               # 131072 contraction rows per core
KT = KTOT // P                   # 1024 K-tiles for head matmul
HW_W = 12 + 64                   # 76 head outputs (v1 | adv), unpadded
NB_H = 128                       # nodes per W2/head block (one A2A quarter)
NBLK = NSLICE // NB_H            # 4 head blocks
SLAB_K = 128                     # k-tiles per head-weight slab (2/blk)

# tile processing order (both layers): quarter-major, so A2A chunk q is
# complete after 8 consecutive tiles and the gather index tables (packed
# in this same order) stay contiguous for batched gathers.
TILE_ORDER = [4 * k + q for q in range(4) for k in range(8)]
MP1_GRP = 1                      # tiles per mp1 gather
MP2_GRP = 1                      # tiles per mp2 gather
N_SWQ = 4                        # SWDGE queues (gathers round-robin)


def _pack_edges(edge_index, edge_weight):
    """Sort edges (+ self loops) by dst, pack into 128-edge chunks such that
    every chunk's dsts fall in one 128-node tile; chunk tables are laid out
    in TILE_ORDER. Returns device tables."""
    src = np.asarray(edge_index[0], np.int64)
    dst = np.asarray(edge_index[1], np.int64)
    ew = np.asarray(edge_weight, np.float32)

    deg = np.zeros(N, np.float32)
    np.add.at(deg, dst, ew)
    deg += 1.0
    dinv = (1.0 / np.sqrt(deg)).astype(np.float32)

    norm = ew * dinv[src] * dinv[dst]
    # self loops: src=dst=n, weight 1/deg[n]
    src_a = np.concatenate([src, np.arange(N, dtype=np.int64)])
    dst_a = np.concatenate([dst, np.arange(N, dtype=np.int64)])
    nrm_a = np.concatenate([norm, dinv * dinv]).astype(np.float32)

    order = np.argsort(dst_a, kind="stable")
    src_a, dst_a, nrm_a = src_a[order], dst_a[order], nrm_a[order]

    src_pk, nrm_pk, off_pk = [], [], []
    tile_nch = {}
    for t in TILE_ORDER:
        sel = (dst_a >= t * P) & (dst_a < (t + 1) * P)
        s, d, w = src_a[sel], dst_a[sel], nrm_a[sel]
        cnt = len(s)
        nch = max(1, (cnt + P - 1) // P)
        pad = nch * P - cnt
        src_pk.append(np.concatenate([s, np.zeros(pad, np.int64)]))
        nrm_pk.append(np.concatenate([w, np.zeros(pad, np.float32)]))
        off_pk.append(np.concatenate([d - t * P, np.zeros(pad, np.int64)]))
        tile_nch[t] = nch

    src_pk = np.concatenate(src_pk)
    nrm_pk = np.concatenate(nrm_pk)
    off_pk = np.concatenate(off_pk)
    e_pad = len(src_pk)
    nchunk = e_pad // P

    # dma_gather index table: logical idx i lives at [i % 16, i // 16]
    gidx = np.zeros((P, e_pad // 16), np.int16)
    for p16 in range(16):
        gidx[p16, :] = src_pk[p16::16].astype(np.int16)
    gidx = np.tile(gidx[:16], (8, 1))  # replicate over all 128 partitions

    # per-chunk column tables: [p, c] = value of edge c*128+p
    nrm_t = nrm_pk.reshape(nchunk, P).T.copy()          # [128, nchunk] f32
    off_t = off_pk.reshape(nchunk, P).T.astype(np.float32).copy()
    return gidx, nrm_t, off_t, tile_nch, nchunk


def _prep_host(inputs):
    """All host-side numpy preprocessing: edge packing, weight layout, batch shard."""
    x = np.asarray(inputs["x"], np.float32)
    gidx, nrm_t, off_t, tile_nch, nchunk = _pack_edges(
        inputs["edge_index"], inputs["edge_weight"]
    )
    wdt = BF16 if BF else np.float32

    W1 = np.asarray(inputs["W1"], np.float32)      # [16,128]
    b1 = np.asarray(inputs["b1"], np.float32)      # [128]
    W2 = np.asarray(inputs["W2"], np.float32)      # [128,256]
    b2 = np.asarray(inputs["b2"], np.float32)      # [256]
    advW = np.asarray(inputs["advW"], np.float32)  # [N*256, 12]
    advb = np.asarray(inputs["advb"], np.float32)
    v1W = np.asarray(inputs["v1W"], np.float32)    # [N*256, 64]
    v1b = np.asarray(inputs["v1b"], np.float32)
    v2W = np.asarray(inputs["v2W"], np.float32)
    v2b = np.asarray(inputs["v2b"], np.float32)
    v3W = np.asarray(inputs["v3W"], np.float32)
    v3b = np.asarray(inputs["v3b"], np.float32)

    # W1 block-diagonal over the 4 local batches, plus a bias row driven by
    # a constant-1 row appended to aggT on device: [65, 512]
    w1bd = np.zeros((BF1 + 1, B_LOC * F1), np.float32)
    for b in range(B_LOC):
        w1bd[b * F_IN:(b + 1) * F_IN, b * F1:(b + 1) * F1] = W1
    w1bd[BF1, :] = np.tile(b1, B_LOC)

    # dueling combine matrix (adv part): out = C.T @ adv + val
    C = np.zeros((12, 12), np.float32)
    for h in range(3):
        for a in range(4):
            i = h * 4 + a
            C[i, i] += 1.0
            for a2 in range(4):
                C[h * 4 + a2, i] -= 0.25

    # small fp32 consts packed into one [128, SC_COLS] blob:
    #   col 0: b2[0:128];  col 1: b2[128:256]
    #   col 2: advb (12) | v1b (64 @ rows 12:76) | v2b (64 @ rows 76:140->wrap)
    # keep it simple: fixed columns, zero-padded rows.
    sc = np.zeros((P, 8), np.float32)
    sc[:, 0] = b2[0:P]
    sc[:, 1] = b2[P:F2]
    sc[0:12, 2] = advb
    sc[0:64, 3] = v1b
    sc[0:64, 4] = v2b
    sc[0:64, 5] = v3W[:, 0]
    sc[0, 6] = v3b[0]

    shared = {
        "gidx": gidx,
        "nrm_t": nrm_t.copy(),
        "off_t": off_t.copy(),
        "w1bd": w1bd.astype(wdt).copy(),
        "w2": W2.astype(wdt).copy(),
        "smallc": sc,
        "v2w": v2W.copy(),                          # [64,64]
        "cmat": C,
    }

    per_core = []
    for j in range(NC_CORES):
        # x batch-shard, node-major rows [N, b, f] -> [N, 64], zero-padded
        # to 128 bf16 cols (dma_gather rows must be a multiple of 256B)
        x_loc = (x[j * B_LOC:(j + 1) * B_LOC].transpose(1, 0, 2)
                 .reshape(N, BF1).astype(wdt))
        if BF:
            x_loc = np.concatenate(
                [x_loc, np.zeros((N, BF1), wdt)], axis=1)
        x_loc = x_loc.copy()
        # head weights for this core's node slice, pre-tiled to
        # [128, KT*76]: col block k holds K-tile k = advW/v1W rows
        # [r0+128k, r0+128k+128), columns = v1 (64) | adv (12).
        r0 = j * KTOT
        aw = advW[r0:r0 + KTOT].reshape(KT, P, 12)
        vw = v1W[r0:r0 + KTOT].reshape(KT, P, 64)
        hw = np.concatenate([vw, aw], axis=2)       # [KT, 128, 76]
        hw_t = hw.transpose(1, 0, 2).reshape(P, KT * HW_W)
        hw_t = hw_t.astype(wdt).copy()
        per_core.append({"x_loc": x_loc, "headw_t": hw_t})

    return shared, per_core, tile_nch, nchunk


# ---------------- device program ----------------

def build_program(nc, tc, tile_nch, nchunk, io, collectives=True, phases=(1,1,1), repeat=1):
    """Emit the Tile program. io: dict of name -> DRAM AP."""
    import concourse.bass as bass
    import concourse.mybir as mybir
    import concourse.tile as tile
    from concourse.masks import make_identity

    f32 = mybir.dt.float32
    bf16 = mybir.dt.bfloat16
    wdt = bf16 if BF else f32
    i16 = mybir.dt.int16
    i32 = mybir.dt.int32
    AF = mybir.ActivationFunctionType
    OP = mybir.AluOpType

    # chunk index ranges per tile, in packed (TILE_ORDER) layout
    tile_c0 = {}
    c = 0
    for t in TILE_ORDER:
        tile_c0[t] = c
        c += tile_nch[t]
    assert c == nchunk

    from contextlib import ExitStack
    with ExitStack() as ctx:
        const = ctx.enter_context(tc.tile_pool(name="const", bufs=1))
        sb = ctx.enter_context(tc.tile_pool(name="sb", bufs=3))
        sb_msg = ctx.enter_context(tc.tile_pool(name="msg", bufs=3))
        sb_s = ctx.enter_context(tc.tile_pool(name="sbs", bufs=3))
        sb_hw = ctx.enter_context(tc.tile_pool(name="sbhw", bufs=3))
        ps_agg = ctx.enter_context(tc.tile_pool(name="ps_agg", bufs=2, space="PSUM"))
        ps_t = ctx.enter_context(tc.tile_pool(name="ps_t", bufs=3, space="PSUM"))
        ps_tr = ctx.enter_context(tc.tile_pool(name="ps_tr", bufs=2, space="PSUM"))
        ps_head = ctx.enter_context(tc.tile_pool(name="ps_head", bufs=1, space="PSUM"))
        dram = ctx.enter_context(tc.tile_pool(name="dram", bufs=1, space="DRAM"))
        for _rep in range(repeat):
            # ---- constants into SBUF
            ident = const.tile([P, P], f32)
            make_identity(nc, ident[:])
            ident_w = ident
            if BF:
                ident_b = const.tile([P, P], wdt)
                nc.vector.tensor_copy(ident_b[:], ident[:])
                ident_w = ident_b
            iota_i = const.tile([P, P], i32)
            nc.gpsimd.iota(iota_i[:], pattern=[[1, P]], base=0, channel_multiplier=0)
            iota_f = const.tile([P, P], f32)
            nc.vector.tensor_copy(iota_f[:], iota_i[:])
            ones1 = const.tile([1, P], f32)
            nc.vector.memset(ones1[:], 1.0)

            gidx_sb = const.tile([P, (nchunk * P) // 16], i16)
            nc.sync.dma_start(gidx_sb[:], io["gidx"][:, :])
            # nrm | off chunk tables in one load
            no_sb = const.tile([P, 2 * nchunk], f32)
            nc.sync.dma_start(no_sb[:, 0:nchunk], io["nrm_t"][:, :])
            nc.sync.dma_start(no_sb[:, nchunk:2 * nchunk], io["off_t"][:, :])
            nrm_sb = no_sb[:, 0:nchunk]
            off_sb = no_sb[:, nchunk:2 * nchunk]

            w1bd_sb = const.tile([BF1 + 1, B_LOC * F1], wdt)
            nc.sync.dma_start(w1bd_sb[:], io["w1bd"][:, :])
            w2_sb = const.tile([P, F2], wdt)
            nc.sync.dma_start(w2_sb[:], io["w2"][:, :])
            smallc = const.tile([P, 8], f32)
            nc.sync.dma_start(smallc[:], io["smallc"][:, :])
            b2a = smallc[:, 0:2]
            advb_sb = smallc[0:12, 2:3]
            v1b_sb = smallc[0:64, 3:4]
            v2b_sb = smallc[0:64, 4:5]
            v3w_sb = smallc[0:64, 5:6]
            v3b_sb = smallc[0:1, 6:7]
            v2w_sb = const.tile([64, 64], f32)
            nc.sync.dma_start(v2w_sb[:], io["v2w"][:, :])
            cmat_sb = const.tile([12, 12], f32)
            nc.sync.dma_start(cmat_sb[:], io["cmat"][:, :])

            # scratch DRAM
            h1_dram = dram.tile([N, BFH], wdt)           # node-major H1
            # agg2 feature-major, one buffer pair per node-quarter so the
            # AllToAll pipelines with mp2 and the head phase:
            # a2a_*_q[q][k, fin, b, n128]  (b-major so the head lhsT slice
            # over (s,b) at fixed n is a single strided free dim)
            a2a_in_q = [dram.tile([NC_CORES, F1, B_LOC, P], wdt,
                                  name=f"a2ain{q}") for q in range(4)]
            a2a_out_q = [dram.tile([NC_CORES, F1, B_LOC, P], wdt,
                                   name=f"a2aout{q}") for q in range(4)]
            ar_in = dram.tile([B, HW_W], f32)
            ar_out = dram.tile([B, HW_W], f32)

            # ================= message passing (shared by both layers) =====
            def mp_layer(x_dram, elem, out_cb, dt, grp, gelem=None):
                """gather + scatter for one GCN layer; out_cb(t, agg_psum_ap).

                Tiles processed in TILE_ORDER, gathered in groups of `grp`
                tiles (contiguous chunk ranges in the packed tables).
                `gelem` = gathered row width (>= elem; rows may be padded to
                satisfy the 256B gather-row minimum).
                One-hot scatter matrices S[c][e, n] = norm[e] * (dstoff[e]==n)
                are built per chunk on DVE."""
                if gelem is None:
                    gelem = elem
                for gi, g0 in enumerate(range(0, NTILES, grp)):
                    tiles = TILE_ORDER[g0:g0 + grp]
                    c0 = tile_c0[tiles[0]]
                    nch_g = sum(tile_nch[t] for t in tiles)
                    nidx = nch_g * P
                    msg = sb_msg.tile([P, nch_g * gelem], dt, tag="msg")
                    nc.gpsimd.dma_gather(
                        out_ap=msg[:].rearrange("p (c e) -> p c e", e=gelem),
                        in_ap=x_dram[:, :],
                        idxs_ap=gidx_sb[:, c0 * 8:(c0 + nch_g) * 8],
                        num_idxs=nidx,
                        num_idxs_reg=nidx,
                        elem_size=gelem,
                        queue_num=gi % N_SWQ,
                    )
                    for t in tiles:
                        cs = range(tile_c0[t], tile_c0[t] + tile_nch[t])
                        nch = tile_nch[t]
                        s_t = sb_s.tile([P, nch * P], dt, tag="sC")
                        for i, c in enumerate(cs):
                            # S[e, n] = (iota[n] == dstoff[e]) * norm[e]
                            nc.vector.tensor_scalar(
                                out=s_t[:, i * P:(i + 1) * P], in0=iota_f[:],
                                scalar1=off_sb[:, c:c + 1], scalar2=nrm_sb[:, c:c + 1],
                                op0=OP.is_equal, op1=OP.mult,
                            )
                        agg = ps_agg.tile([P, elem], f32, tag="agg")
                        for i, c in enumerate(cs):
                            mc = c - c0
                            nc.tensor.matmul(
                                agg[:],
                                lhsT=s_t[:, i * P:(i + 1) * P],
                                rhs=msg[:, mc * gelem:mc * gelem + elem],
                                start=(i == 0),
                                stop=(i == nch - 1),
                            )
                        out_cb(t, agg)

            # ---- layer 1
            def l1_out(t, agg):
                # transpose agg [128n, 64] -> aggT [64, 128n]
                agg_sb = sb.tile([P, BF1], wdt, tag="agg1sb")
                nc.vector.tensor_copy(agg_sb[:], agg[:])
                psTf = ps_tr.tile([P, P], wdt, tag="workT", name="psTf")
                psT = psTf[0:BF1, :]
                nc.tensor.transpose(psT, agg_sb[:], ident_w[:])
                aggT = sb.tile([BF1 + 1, P], wdt, tag="aggT1")
                nc.vector.tensor_copy(aggT[0:BF1, :], psT)
                nc.vector.memset(aggT[BF1:BF1 + 1, :], 1.0)
                # H1[t] = relu(aggT_aug.T @ w1bd_aug)  (last row carries b1)
                psH = ps_t.tile([P, B_LOC * F1], f32, tag="work")
                nc.tensor.matmul(psH[:], lhsT=aggT[:], rhs=w1bd_sb[:], start=True, stop=True)
                h1sb = sb.tile([P, BFH], wdt, tag="h1sb")
                nc.scalar.activation(h1sb[:], psH[:], AF.Relu)
                nc.sync.dma_start(h1_dram[t * P:(t + 1) * P, :], h1sb[:])

            if phases[0]:
                with nc.named_scope("mp1"):
                    mp_layer(io["x_loc"], BF1, l1_out, wdt, MP1_GRP,
                             gelem=(2 * BF1 if BF else BF1))

            # ---- layer 2 message passing -> a2a_in (feature-major [k, fin, n', b]
            # so the post-A2A W2-rhs reads are contiguous per partition)
            def l2_out(t, agg):
                agg_sb = sb.tile([P, BFH], wdt, tag="agg2sb")
                nc.vector.tensor_copy(agg_sb[:], agg[:])
                a2a_sb = sb.tile([P, BFH], wdt, tag="a2asb")
                a2a_3d = a2a_sb[:].rearrange("f (b n) -> f b n", b=B_LOC)
                for b in range(B_LOC):
                    psT = ps_tr.tile([P, P], wdt, tag="workT")
                    nc.tensor.transpose(psT[:], agg_sb[:, b * F1:(b + 1) * F1], ident_w[:])
                    nc.vector.tensor_copy(a2a_3d[:, b, :], psT[:])
                k, q = t // 4, t % 4
                nc.sync.dma_start(a2a_in_q[q][k, :, :, :], a2a_sb[:])

            if phases[1]:
                with nc.named_scope("mp2"):
                    mp_layer(h1_dram, BFH, l2_out, wdt, MP2_GRP)

            # ====== per-quarter AllToAll chunk + W2 + head partials ======
            SB_COLS = NB_H * B_LOC                       # 256 cols per src core
            ps_hd = ps_head.tile([P, HW_W], f32)
            nblocks = NBLK if phases[2] else 0
            # all head matmuls accumulate with start=False onto a zeroed
            # tile (interleaved per-col-group start groups confuse the
            # sim's 2KB-granular PSUM pending-zero tracking)
            nc.vector.memset(ps_hd[:], 0.0)

            def a2a_chunk(q):
                if collectives:
                    nc.gpsimd.collective_compute(
                        "AllToAll",
                        mybir.AluOpType.bypass,
                        replica_groups=[list(range(NC_CORES))],
                        ins=[a2a_in_q[q][:].opt()],
                        outs=[a2a_out_q[q][:].opt()],
                    )
                else:
                    for s in range(NC_CORES):
                        nc.sync.dma_start(
                            a2a_out_q[q][s].rearrange("f b n -> f (b n)"),
                            a2a_in_q[q][s].rearrange("f b n -> f (b n)"),
                        )

            for nb in range(nblocks):
                q = nb
                with nc.named_scope(f"a2a{q}"):
                    a2a_chunk(q)
                _sid, _ = nc.enter_named_scope(f"hd{nb}", False)
                # head-weight slabs for this block: 2 x 128 k-tiles
                hw_sbs = []
                for hh in range(2):
                    hw_sb = sb_hw.tile([P, SLAB_K * HW_W], wdt, tag="hwslab",
                                       name=f"hwslab{hh}")
                    k0 = (nb * 2 + hh) * SLAB_K * HW_W
                    nc.scalar.dma_start(
                        hw_sb[:], io["headw_t"][:, k0:k0 + SLAB_K * HW_W])
                    hw_sbs.append(hw_sb)
                # stage rhs [128 fin, (s, b, n)] in ONE dma: contiguous 1KB
                # runs per (partition, src) in a2a_out
                rhs_sb = sb.tile([P, NC_CORES * SB_COLS], wdt, tag="w2rhs")
                nc.sync.dma_start(
                    rhs_sb[:].rearrange("f (s b n) -> f s b n",
                                        s=NC_CORES, b=B_LOC),
                    a2a_out_q[q][:, :, :, :].rearrange("s f b n -> f s b n"),
                )
                # W2 rhs read permuted to (n, s, b) column order so h2's 32
                # batch-cols per node come out CONTIGUOUS (fast ldweights in
                # the flipped head matmul below)
                rhs_nsb = (rhs_sb[:]
                           .rearrange("f (s b n) -> f s b n",
                                      s=NC_CORES, b=B_LOC)
                           .rearrange("f s b n -> f n s b"))
                NQ = 16                                  # nodes per W2 matmul
                h2 = []
                for fh in range(2):
                    h2sb = sb.tile([P, NC_CORES * SB_COLS], wdt, tag="h2sb")
                    for qq in range(NB_H // NQ):  # free split: 512-col matmuls
                        sl = slice(qq * NQ * B, (qq + 1) * NQ * B)
                        psW = ps_t.tile([P, 512], f32, tag="work")
                        nc.tensor.matmul(
                            psW[:], lhsT=w2_sb[:, fh * P:(fh + 1) * P],
                            rhs=rhs_nsb[:, qq * NQ:(qq + 1) * NQ, :, :],
                            start=True, stop=True,
                        )
                        # relu + per-partition bias b2[fh*128 + p]
                        nc.scalar.activation(h2sb[:, sl], psW[:], AF.Relu,
                                             bias=b2a[:, fh:fh + 1])
                    # cols (n, s, b): node i's 32 batch-cols contiguous
                    h2.append(h2sb[:].rearrange("p (n x) -> p n x", n=NB_H))
                # flipped head contraction, col-tiled over 4 PE col groups:
                # ps_hd[32j:32j+32, :] += h2_k[128,32].T @ hw_k[128,76]
                for i in range(NB_H):
                    for fh in range(2):
                        kk = 2 * i + fh               # k-tile index in block
                        j = kk % 4
                        hs, hc = kk // SLAB_K, kk % SLAB_K
                        nc.tensor.matmul(
                            ps_hd[32 * j:32 * (j + 1), :],
                            lhsT=h2[fh][:, i, :],
                            rhs=hw_sbs[hs][:, hc * HW_W:(hc + 1) * HW_W],
                            start=False,
                            stop=(nb == nblocks - 1) and (kk >= 2 * NB_H - 4),
                            tile_position=(0, 32 * j),
                            skip_group_check=True,
                        )
                nc.leave_named_scope(f"hd{nb}", _sid, False)

            # sum the 4 col-group partial slices -> [32, 76]
            # (chained: only one PSUM operand allowed per DVE instruction)
            pa = sb.tile([B, HW_W], f32, tag="pa")
            nc.vector.tensor_copy(pa[:], ps_hd[0:32, :])
            pb = sb.tile([B, HW_W], f32, tag="pb")
            nc.vector.scalar_tensor_tensor(
                pb[:], pa[:], 1.0, ps_hd[32:64, :], OP.mult, OP.add)
            pc = sb.tile([B, HW_W], f32, tag="pc")
            nc.vector.scalar_tensor_tensor(
                pc[:], pb[:], 1.0, ps_hd[64:96, :], OP.mult, OP.add)
            part_sb = sb.tile([B, HW_W], f32, tag="part")
            nc.vector.scalar_tensor_tensor(
                part_sb[:], pc[:], 1.0, ps_hd[96:128, :], OP.mult, OP.add)
            nc.sync.dma_start(ar_in[:, :], part_sb[:])

            # ================= AllReduce partials =================
            if collectives:
                nc.gpsimd.collective_compute(
                    "AllReduce",
                    mybir.AluOpType.add,
                    replica_groups=[list(range(NC_CORES))],
                    ins=[ar_in[:].opt()],
                    outs=[ar_out[:].opt()],
                )
            else:
                nc.sync.dma_start(ar_out[:, :], ar_in[:, :])
            red_sb = sb.tile([B, HW_W], f32, tag="red")
            nc.sync.dma_start(red_sb[:], ar_out[:, :])
            # transpose [32 b, 76] -> [76, 32 b] for the tail matmuls
            psRf = ps_t.tile([P, 512], f32, tag="work", name="psRf")
            psR = psRf[0:HW_W, 0:B]
            nc.tensor.transpose(psR, red_sb[:], ident[0:B, 0:B])
            redT = sb.tile([HW_W, B], f32, tag="redT")
            nc.vector.tensor_copy(redT[:], psR[:])

            # ================= final MLP + dueling combine =================
            adv_sb = sb.tile([12, B], f32, tag="adv")
            nc.scalar.activation(adv_sb[:], redT[64:76, :], AF.Relu, bias=advb_sb)
            # val path
            v1_sb = sb.tile([64, B], f32, tag="v1")
            nc.scalar.activation(v1_sb[:], redT[0:64, :], AF.Relu, bias=v1b_sb)
            psV = ps_t.tile([64, B], f32, tag="work")
            nc.tensor.matmul(psV[:], lhsT=v2w_sb[:], rhs=v1_sb[:], start=True, stop=True)
            v2_sb = sb.tile([64, B], f32, tag="v2")
            nc.scalar.activation(v2_sb[:], psV[:], AF.Relu, bias=v2b_sb)
            psV3 = ps_t.tile([1, B], f32, tag="work")
            nc.tensor.matmul(psV3[:], lhsT=v3w_sb, rhs=v2_sb[:], start=True, stop=True)
            val_sb = sb.tile([1, B], f32, tag="val")
            nc.vector.tensor_scalar_add(val_sb[:], psV3[:], v3b_sb)
            # out = cmat.T @ adv + 1.T @ val
            psO = ps_t.tile([12, B], f32, tag="work")
            nc.tensor.matmul(psO[:], lhsT=cmat_sb[:], rhs=adv_sb[:], start=True, stop=False)
            nc.tensor.matmul(psO[:], lhsT=ones1[:, 0:12], rhs=val_sb[:], start=False, stop=True)
            out_sb = sb.tile([12, B], f32, tag="out")
            nc.vector.tensor_copy(out_sb[:], psO[:])
            nc.sync.dma_start(io["out"][:, :], out_sb[:])


# ---------------- driver ----------------

LAST_RESULTS = None

def _input_specs(shared, per_core):
    """name -> (shape, np dtype); per-core entries use per_core[0] shapes."""
    specs = {}
    for k, v in shared.items():
        specs[k] = v
    for k, v in per_core[0].items():
        specs[k] = v
    return specs


def kernel(**inputs) -> np.ndarray:
    import concourse.bacc as bacc
    import concourse.mybir as mybir
    import concourse.tile as tile
    from concourse import bass_utils

    shared, per_core, tile_nch, nchunk = _prep_host(inputs)

    nc = bacc.Bacc("TRN2", target_bir_lowering=False, debug=False,
                   enable_asserts=False, num_devices=NC_CORES,
                   num_swdge_queues=N_SWQ)

    io = {}
    specs = _input_specs(shared, per_core)
    for name, arr in specs.items():
        io[name] = nc.dram_tensor(
            name, list(arr.shape), mybir.dt.from_np(arr.dtype), kind="ExternalInput"
        ).ap()
    io["out"] = nc.dram_tensor(
        "out", [12, B], mybir.dt.float32, kind="ExternalOutput"
    ).ap()

    with tile.TileContext(nc) as tc:
        build_program(nc, tc, tile_nch, nchunk, io)
    nc.compile()

    in_maps = []
    for j in range(NC_CORES):
        m = dict(shared)
        m.update(per_core[j])
        in_maps.append(m)

    res = bass_utils.run_bass_kernel_spmd(
        nc, in_maps, core_ids=list(range(NC_CORES)),
    )
    global LAST_RESULTS
    LAST_RESULTS = res
    out = res.results[0]["out"]                      # [12, 32]
    return out.T.reshape(B, 3, 4).copy().astype(np.float32)


if __name__ == "__main__":
    rng = np.random.default_rng(0)
    ei = rng.integers(0, N, (2, E)).astype(np.int64)
    demo = {
        "x": rng.standard_normal((B, N, F_IN), np.float32),
        "edge_index": ei,
        "edge_weight": rng.random(E, np.float32),
        "W1": rng.standard_normal((F_IN, F1), np.float32) / 4,
        "b1": np.zeros(F1, np.float32),
        "W2": rng.standard_normal((F1, F2), np.float32) / 11.3,
        "b2": np.zeros(F2, np.float32),
        "advW": rng.standard_normal((N * F2, 12), np.float32) / 1024,
        "advb": np.zeros(12, np.float32),
        "v1W": rng.standard_normal((N * F2, 64), np.float32) / 1024,
        "v1b": np.zeros(64, np.float32),
        "v2W": rng.standard_normal((64, 64), np.float32) / 8,
        "v2b": np.zeros(64, np.float32),
        "v3W": rng.standard_normal((64, 1), np.float32) / 8,
        "v3b": np.zeros(1, np.float32),
    }
    print(kernel(**demo).shape)
